# revision 6
# baseline (speedup 1.0000x reference)
"""Trainium2 Bass kernel for nn_Attention_28862180229481.

Attention with learned relative-position bias:
  qkv = x @ qkv_w.T ; q,k,v per head
  pos = einsum('nmp,hp->hnm', pos_emb, pos_proj_w)
  attn = softmax((q@k.T + pos) * scale); out = (attn @ v) @ proj_w.T + proj_b

Sharding: data-parallel over batch (16 batches -> 8 cores x 2).
pos bias is m-sharded: core r computes pos[:, :, r*99:(r+1)*99] (via a
DMA-xbar transpose of pos_emb into [p, n*m] layout + K=48 matmul), stores
it unscaled as fp8e5, AllGathers across the 8 cores, and every core then
consumes the full [12,785,785] bias in fp8 during its local attention.

Softmax: logits are bounded (~N(0,0.31) after scale) so no max-subtraction:
probs = exp(scale*(qk+pos)); row-sum comes free via a ones-column packed
next to V in the attn@v matmul; normalization folds into the PSUM eviction.
"""

import numpy as np

import concourse.bass as bass
import concourse.mybir as mybir
import concourse.tile as tile
from concourse import bacc
from concourse.bass_utils import run_bass_kernel_spmd

# problem shapes
B, N, C, H, HD, P = 16, 785, 768, 12, 64, 48
NCORES = 8
BL = B // NCORES          # 2 local batches
TOK = BL * N              # 1570
TOKP = 1600               # padded tokens for xbar transpose (mult of 32)
MS = 100                  # m-shard size (8*100 = 800 >= 785)
PP = 64                   # host-padded p dim (48 -> 64)
SCALE = HD ** -0.5
CK = C // 128             # 6 contraction chunks of 128
XMM = MS * N              # 78500 m-major flat size of one pos shard
XMM_P = 78848             # padded to mult of 512 for the collective
# n-range chunks for the pos pipeline (posembT SBUF residency = 99*nr elems)
N_RANGES = [(0, 256), (256, 512), (512, 768), (768, 785)]

f32 = mybir.dt.float32
bf16 = mybir.dt.bfloat16
fp8 = mybir.dt.float8e5
Exp = mybir.ActivationFunctionType.Exp
Copy = mybir.ActivationFunctionType.Copy
ADD = mybir.AluOpType.add

_cache = {}


def _mm_chunks(nc, psum, lhsT, rhs, start, stop, fmax=512):
    """matmul split along the moving free dim into <=512 chunks."""
    F = rhs.shape[-1]
    j = 0
    while j < F:
        je = min(j + fmax, F)
        nc.tensor.matmul(psum[:, j:je], lhsT, rhs[:, j:je], start=start, stop=stop)
        j = je


def build():
    nc = bacc.Bacc(
        "TRN2", target_bir_lowering=False, debug=False, num_devices=NCORES
    )

    # ---- I/O -------------------------------------------------------------
    x_in = nc.dram_tensor("x", [BL, N, C], f32, kind="ExternalInput").ap()
    qkvw_in = nc.dram_tensor("qkv_w", [3 * C, C], f32, kind="ExternalInput").ap()
    pos_in = nc.dram_tensor("pos_slice", [N, MS, PP], f32, kind="ExternalInput").ap()
    wp_in = nc.dram_tensor("pos_proj_w", [H, P], f32, kind="ExternalInput").ap()
    projw_in = nc.dram_tensor("proj_w", [C, C], f32, kind="ExternalInput").ap()
    projb_in = nc.dram_tensor("proj_b", [C], f32, kind="ExternalInput").ap()
    y_out = nc.dram_tensor("out", [BL, N, C], f32, kind="ExternalOutput").ap()

    # ---- internal DRAM ---------------------------------------------------
    xb = nc.dram_tensor("xb", [TOKP, C], bf16).ap()            # bf16 x
    qkvwb = nc.dram_tensor("qkvwb", [3 * C, C], bf16).ap()
    projwb = nc.dram_tensor("projwb", [C, C], bf16).ap()
    projbb = nc.dram_tensor("projbb", [1, C], bf16).ap()
    wpb = nc.dram_tensor("wpb", [H, P], bf16).ap()
    # pos bounce: m-parity-packed rows of 128 (two padded-64 p-rows each)
    XPAD = sum(-(-(n1 - n0) * MS // 2 // 16) * 16 for n0, n1 in N_RANGES)
    posb = nc.dram_tensor("posb", [XPAD, 128], bf16).ap()
    pos8_loc = nc.dram_tensor("pos8_loc", [H, XMM_P], fp8).ap()
    pos8_all = nc.dram_tensor(
        "pos8_all", [NCORES, H, XMM_P], fp8, addr_space="Shared"
    ).ap()

    with tile.TileContext(nc) as tc:
        kernel_body(
            nc, tc, x_in, qkvw_in, pos_in, wp_in, projw_in, projb_in, y_out,
            xb, qkvwb, projwb, projbb, wpb, posb, pos8_loc, pos8_all,
        )
    nc.compile()
    return nc


def kernel_body(nc, tc, x_in, qkvw_in, pos_in, wp_in, projw_in, projb_in,
                y_out, xb, qkvwb, projwb, projbb, wpb, posb, pos8_loc,
                pos8_all):
    from contextlib import ExitStack

    ms_last = N - 7 * MS  # 85 valid rows in the last shard

    with ExitStack() as stk:
        const = stk.enter_context(tc.tile_pool(name="const", bufs=1))
        wTd = const.tile([128, H], bf16)       # pos_proj_w.T at bases 0 and 64
        ones_mm = const.tile([1, 128], bf16)   # lhsT for bias broadcast
        projb_sb = const.tile([1, C], bf16)
        nc.any.memset(ones_mm[:], 1.0)

        # ============ phase 0: pos pipeline -> AllGather ==================
        # cast pos_emb slice f32->bf16 into padded bounce rows
        nc.gpsimd.dma_start(out=wpb[:, :], in_=wp_in[:, :])  # cast H x P
        nc.sync.dma_start(out=wTd[0:P, :], in_=wpb.rearrange("h p -> p h"))
        nc.sync.dma_start(out=wTd[64:64 + P, :], in_=wpb.rearrange("h p -> p h"))

        row0 = 0
        chunk_info = []  # (row0, rows_pad, n0, n1)
        for n0, n1 in N_RANGES:
            rows = (n1 - n0) * MS // 2
            rows_pad = -(-rows // 16) * 16
            nc.gpsimd.dma_start(
                out=posb[row0:row0 + rows, :],
                in_=pos_in[n0:n1].rearrange("n m p -> (n m) p")
                .rearrange("(r two) p -> r (two p)", two=2),
            )
            chunk_info.append((row0, rows_pad, n0, n1))
            row0 += rows_pad

        with ExitStack() as pstk:
            ppool = pstk.enter_context(tc.tile_pool(name="posT", bufs=2))
            ppsum = pstk.enter_context(
                tc.tile_pool(name="pos_ps", bufs=4, space="PSUM"))
            pacc = pstk.enter_context(tc.tile_pool(name="pos_acc", bufs=2))

            for row0, rows_pad, n0, n1 in chunk_info:
                nr = n1 - n0
                pT = ppool.tile([128, rows_pad], bf16, tag="posT", name="posT")
                nc.sync.dma_start(
                    out=pT[:, :], in_=posb[row0:row0 + rows_pad, :],
                    transpose=True,
                )
                # per parity: [p, m2, n] views of this n-range
                pTe = pT[0:P, 0:nr * MS // 2].rearrange(
                    "p (n m2) -> p m2 n", m2=MS // 2)
                pTo = pT[64:64 + P, 0:nr * MS // 2].rearrange(
                    "p (n m2) -> p m2 n", m2=MS // 2)
                for mg in range(10):          # 100 = 10 groups of 10
                    acc = pacc.tile([H, 10 * N], fp8, tag="pacc", name="pacc")
                    for mi in range(10):
                        m = mg * 10 + mi
                        par, m2 = m % 2, m // 2
                        ps = ppsum.tile([H, 512], f32, tag="pps", name="pps")
                        nc.tensor.matmul(
                            ps[:, 0:nr],
                            wTd[64 * par:64 * par + P, :],
                            (pTo if par else pTe)[:, m2, :],
                            start=True, stop=True,
                        )
                        nc.scalar.activation(
                            acc[:, mi * N + n0: mi * N + n1], ps[:, 0:nr], Copy)
                    nc.sync.dma_start(
                        out=pos8_loc[:, mg * 10 * N:(mg + 1) * 10 * N]
                        .rearrange("h (m n) -> h m n", m=10)[:, :, n0:n1],
                        in_=acc[:, 0:10 * N]
                        .rearrange("h (m n) -> h m n", m=10)[:, :, n0:n1],
                    )

        nc.gpsimd.collective_compute(
            "AllGather", mybir.AluOpType.bypass,
            replica_groups=[list(range(NCORES))],
            ins=[pos8_loc[:, :]], outs=[pos8_all[:, :, :]],
        )

        # ============ phase 1: weight/x prep (overlaps AllGather) =========
        nc.gpsimd.dma_start(out=qkvwb[:, :], in_=qkvw_in[:, :])
        nc.gpsimd.dma_start(out=projwb[:, :], in_=projw_in[:, :])
        nc.gpsimd.dma_start(out=projbb[0, :], in_=projb_in[:])
        nc.sync.dma_start(out=projb_sb[:, :], in_=projbb[:, :])
        nc.gpsimd.dma_start(
            out=xb[0:TOK, :], in_=x_in.rearrange("b n c -> (b n) c"))
        zpad = const.tile([TOKP - TOK, C], bf16)
        nc.any.memset(zpad[:], 0.0)
        nc.sync.dma_start(out=xb[TOK:TOKP, :], in_=zpad[:, :])

        wpool = stk.enter_context(tc.tile_pool(name="weights", bufs=1))
        qkvwT = []   # 6 tiles [128, 2304]
        projwT = []  # 6 tiles [128, 768]
        xT = []      # 6 tiles [128, TOKP]
        for c in range(CK):
            t = wpool.tile([128, 3 * C], bf16, tag=f"qkvwT{c}", name=f"qkvwT{c}")
            nc.sync.dma_start(
                out=t[:, :], in_=qkvwb[:, c * 128:(c + 1) * 128],
                transpose=True)
            qkvwT.append(t)
            t = wpool.tile([128, C], bf16, tag=f"projwT{c}", name=f"projwT{c}")
            nc.sync.dma_start(
                out=t[:, :], in_=projwb[:, c * 128:(c + 1) * 128],
                transpose=True)
            projwT.append(t)
            t = wpool.tile([128, TOKP], bf16, tag=f"xT{c}", name=f"xT{c}")
            nc.sync.dma_start(
                out=t[:, :], in_=xb[:, c * 128:(c + 1) * 128], transpose=True)
            xT.append(t)

        # ============ phase 2: qkv projection =============================
        qkT = []  # 12 tiles [128, TOK] rows of (q;k).T
        vag = {}  # (b, r) -> [ms, H*(HD+1)] v with ones column
        with ExitStack() as qstk:
            qpool = qstk.enter_context(tc.tile_pool(name="qkv_sb", bufs=1))
            qpsum_stk = ExitStack()
            qpsum = qpsum_stk.enter_context(
                tc.tile_pool(name="qkv_ps", bufs=3, space="PSUM"))
            for mo in range(12):
                t = qpool.tile([128, TOK], bf16, tag=f"qkT{mo}", name=f"qkT{mo}")
                for j0 in range(0, TOK, 512):
                    j1 = min(j0 + 512, TOK)
                    ps = qpsum.tile([128, 512], f32, tag="qk_ps", name="qk_ps")
                    for c in range(CK):
                        nc.tensor.matmul(
                            ps[:, 0:j1 - j0],
                            qkvwT[c][:, mo * 128:(mo + 1) * 128],
                            xT[c][:, j0:j1],
                            start=(c == 0), stop=(c == CK - 1),
                        )
                    nc.scalar.activation(t[:, j0:j1], ps[:, 0:j1 - j0], Copy)
                qkT.append(t)
            # v with the ones column appended per head: [tok, H, HD+1]
            for b in range(BL):
                for r in range(8):
                    ms = MS if r < 7 else ms_last
                    vt = qpool.tile([MS, H * (HD + 1)], bf16, tag=f"vag{b}_{r}", name=f"vag{b}_{r}")
                    nc.any.memset(vt[:], 1.0)
                    t0 = b * N + r * MS
                    for half in range(2):  # v cols 1536:2048, 2048:2304
                        ps = qpsum.tile([MS, 512], f32, tag="v_ps", name="v_ps")
                        w0 = 1536 + half * 512
                        w1 = min(w0 + 512, 2304)
                        for c in range(CK):
                            nc.tensor.matmul(
                                ps[0:ms, 0:w1 - w0],
                                xT[c][:, t0:t0 + ms],
                                qkvwT[c][:, w0:w1],
                                start=(c == 0), stop=(c == CK - 1),
                            )
                        hh0 = half * 8
                        nhh = (w1 - w0) // HD
                        nc.scalar.activation(
                            vt[0:ms].rearrange("m (h d) -> m h d", h=H)
                            [:, hh0:hh0 + nhh, 0:HD],
                            ps[0:ms, 0:w1 - w0].rearrange(
                                "m (h d) -> m h d", d=HD),
                            Copy,
                        )
                    vag[(b, r)] = vt

            qpsum_stk.close()

            # ============ phase 3: attention ==============================
            apool = qstk.enter_context(tc.tile_pool(name="attn_sb", bufs=1))
            aoT = {}  # (b, ct) -> [128, N] bf16 attn_out.T
            for b in range(BL):
                for ct in range(CK):
                    aoT[(b, ct)] = apool.tile([128, N], bf16, tag=f"aoT{b}_{ct}", name=f"aoT{b}_{ct}")

            dpool = qstk.enter_context(tc.tile_pool(name="attn_dyn", bufs=3))
            p8pool = qstk.enter_context(tc.tile_pool(name="p8", bufs=10))
            apsum_stk = ExitStack()
            spsum = apsum_stk.enter_context(
                tc.tile_pool(name="s_ps", bufs=2, space="PSUM"))
            opsum = apsum_stk.enter_context(
                tc.tile_pool(name="o_ps", bufs=2, space="PSUM"))

            for h in range(12):
                kt = qkT[6 + h // 2]
                ko = 64 * (h % 2)
                qt = qkT[h // 2]
                qo = 64 * (h % 2)
                p8 = []
                for r in range(8):
                    ms = MS if r < 7 else ms_last
                    t = p8pool.tile([MS, N], fp8, tag="p8t", name="p8t")
                    nc.sync.dma_start(
                        out=t[0:ms, :],
                        in_=pos8_all[r, h, 0:ms * N]
                        .rearrange("(m n) -> m n", n=N),
                    )
                    p8.append(t)
                for b in range(BL):
                    po = opsum.tile([HD + 1, N], f32, tag="o_ps", name="o_ps")
                    for r in range(8):
                        ms = MS if r < 7 else ms_last
                        m0 = b * N + r * MS
                        ps = spsum.tile([MS, N], f32, tag="s_ps", name="s_ps")
                        _mm_chunks(
                            nc, ps[0:ms],
                            kt[ko:ko + HD, m0:m0 + ms],
                            qt[qo:qo + HD, b * N:(b + 1) * N],
                            start=True, stop=True,
                        )
                        sm = dpool.tile([MS, N], f32, tag="sum_sb", name="sum_sb")
                        nc.vector.tensor_add(
                            sm[0:ms], ps[0:ms], p8[r][0:ms])
                        pb = dpool.tile([MS, N], bf16, tag="probsT", name="probsT")
                        nc.scalar.activation(
                            pb[0:ms], sm[0:ms], Exp, scale=SCALE)
                        _mm_chunks(
                            nc, po,
                            vag[(b, r)][0:ms]
                            .rearrange("m (h d) -> m h d", h=H)[:, h, :],
                            pb[0:ms],
                            start=(r == 0), stop=(r == 7),
                        )
                    rec = dpool.tile([1, N], f32, tag="recip", name="recip")
                    nc.vector.reciprocal(rec[:, :], po[HD:HD + 1, :])
                    recb = dpool.tile([HD, N], f32, tag="recb", name="recb")
                    nc.gpsimd.partition_broadcast(recb[:, :], rec[:, :])
                    ct, co = (h * HD) // 128, (h * HD) % 128
                    nc.vector.tensor_mul(
                        aoT[(b, ct)][co:co + HD, :], po[0:HD, :], recb[:, :])

            apsum_stk.close()

            # ============ phase 4: output projection ======================
            ypsum = qstk.enter_context(
                tc.tile_pool(name="y_ps", bufs=2, space="PSUM"))
            ypool = qstk.enter_context(tc.tile_pool(name="y_sb", bufs=3))
            for b in range(BL):
                for to in range(7):
                    t0 = to * 128
                    t1 = min(t0 + 128, N)
                    tw = t1 - t0
                    ps = ypsum.tile([128, C], f32, tag="y_ps", name="y_ps")
                    for j0 in (0, 512):
                        j1 = min(j0 + 512, C)
                        for c in range(CK):
                            nc.tensor.matmul(
                                ps[0:tw, j0:j1],
                                aoT[(b, c)][:, t0:t1],
                                projwT[c][:, j0:j1],
                                start=(c == 0), stop=False,
                            )
                        nc.tensor.matmul(
                            ps[0:tw, j0:j1], ones_mm[:, 0:tw],
                            projb_sb[:, j0:j1], start=False, stop=True,
                        )
                    ys = ypool.tile([128, C], f32, tag="y_sb", name="y_sb")
                    nc.scalar.activation(ys[0:tw], ps[0:tw], Copy)
                    nc.sync.dma_start(
                        out=y_out[b, t0:t1, :], in_=ys[0:tw])


def kernel(**inputs):
    x = np.ascontiguousarray(np.asarray(inputs["x"], dtype=np.float32))
    qkv_w = np.ascontiguousarray(np.asarray(inputs["qkv_w"], np.float32))
    pos_emb = np.ascontiguousarray(np.asarray(inputs["pos_emb"], np.float32))
    wp = np.ascontiguousarray(np.asarray(inputs["pos_proj_w"], np.float32))
    proj_w = np.ascontiguousarray(np.asarray(inputs["proj_w"], np.float32))
    proj_b = np.ascontiguousarray(np.asarray(inputs["proj_b"], np.float32))

    if "nc" not in _cache:
        _cache["nc"] = build()
    nc = _cache["nc"]

    pos_pad = np.zeros((N, NCORES * MS, PP), np.float32)
    pos_pad[:, :N, :P] = pos_emb
    in_maps = []
    for i in range(NCORES):
        in_maps.append({
            "x": np.ascontiguousarray(x[i * BL:(i + 1) * BL]),
            "qkv_w": qkv_w,
            "pos_slice": np.ascontiguousarray(
                pos_pad[:, i * MS:(i + 1) * MS, :]),
            "pos_proj_w": wp,
            "proj_w": proj_w,
            "proj_b": proj_b,
        })
    res = run_bass_kernel_spmd(nc, in_maps, core_ids=list(range(NCORES)))
    _cache["last_res"] = res
    out = np.concatenate([res.results[i]["out"] for i in range(NCORES)], axis=0)
    return out.astype(np.float32)


if __name__ == "__main__":
    import reference
    inp = {k: np.asarray(v) for k, v in reference.setup_inputs().items()}
    got = kernel(**inp)
    exp = np.asarray(reference.reference(**inp))
    err = np.abs(got - exp).max() / (np.abs(exp).max() + 1e-9)
    print("rel err:", err)


# revision 15
# speedup vs baseline: 142.3990x; 142.3990x over previous
"""Trainium2 Bass kernel for nn_Attention_28862180229481.

Attention with learned relative-position bias:
  qkv = x @ qkv_w.T ; q,k,v per head
  pos = einsum('nmp,hp->hnm', pos_emb, pos_proj_w)
  attn = softmax((q@k.T + pos) * scale); out = (attn @ v) @ proj_w.T + proj_b

Sharding: data-parallel over batch (16 batches -> 8 cores x 2).
pos bias is m-sharded: core r computes pos[:, :, r*99:(r+1)*99] (via a
DMA-xbar transpose of pos_emb into [p, n*m] layout + K=48 matmul), stores
it unscaled as fp8e5, AllGathers across the 8 cores, and every core then
consumes the full [12,785,785] bias in fp8 during its local attention.

Softmax: logits are bounded (~N(0,0.31) after scale) so no max-subtraction:
probs = exp(scale*(qk+pos)); row-sum comes free via a ones-column packed
next to V in the attn@v matmul; normalization folds into the PSUM eviction.
"""

import numpy as np

import concourse.bass as bass
import concourse.mybir as mybir
import concourse.tile as tile
from concourse import bacc
from concourse.bass_utils import run_bass_kernel_spmd
from concourse.masks import make_identity

# problem shapes
B, N, C, H, HD, P = 16, 785, 768, 12, 64, 48
NCORES = 8
BL = B // NCORES          # 2 local batches
TOK = BL * N              # 1570
TOKP = 1600               # padded tokens for xbar transpose (mult of 32)
MS = 100                  # m-shard size (8*100 = 800 >= 785)
PP = 64                   # host-padded p dim (48 -> 64)
SCALE = HD ** -0.5
CK = C // 128             # 6 contraction chunks of 128
XMM = MS * N              # 78500 m-major flat size of one pos shard
XMM_P = 78848             # padded to mult of 512 for the collective
# n-range chunks for the pos pipeline (posembT SBUF residency = 99*nr elems)
N_RANGES = [(0, 392), (392, 785)]

f32 = mybir.dt.float32
bf16 = mybir.dt.bfloat16
fp16 = mybir.dt.float16
fp8 = mybir.dt.float8e5
Exp = mybir.ActivationFunctionType.Exp
Copy = mybir.ActivationFunctionType.Copy
ADD = mybir.AluOpType.add

_cache = {}


def _mm_chunks(nc, psum, lhsT, rhs, start, stop, fmax=512):
    """matmul split along the moving free dim into <=512 chunks."""
    F = rhs.shape[-1]
    j = 0
    while j < F:
        je = min(j + fmax, F)
        nc.tensor.matmul(psum[:, j:je], lhsT, rhs[:, j:je], start=start, stop=stop)
        j = je


def build(sim_mode=False):
    nc = bacc.Bacc(
        "TRN2", target_bir_lowering=False, debug=False, num_devices=NCORES
    )

    # ---- I/O -------------------------------------------------------------
    x_in = nc.dram_tensor("x", [BL, N, C], f32, kind="ExternalInput").ap()
    qkvw_in = nc.dram_tensor("qkv_w", [3 * C, C], f32, kind="ExternalInput").ap()
    pos_in = nc.dram_tensor("pos_slice", [N, MS, PP], f32, kind="ExternalInput").ap()
    wp_in = nc.dram_tensor("pos_proj_w", [H, P], f32, kind="ExternalInput").ap()
    projw_in = nc.dram_tensor("proj_w", [C, C], f32, kind="ExternalInput").ap()
    projb_in = nc.dram_tensor("proj_b", [C], f32, kind="ExternalInput").ap()
    y_out = nc.dram_tensor("out", [BL, N, C], f32, kind="ExternalOutput").ap()

    # ---- internal DRAM ---------------------------------------------------
    xb = nc.dram_tensor("xb", [TOKP, C], bf16).ap()            # bf16 x
    qkvwb = nc.dram_tensor("qkvwb", [3 * C, C], bf16).ap()
    projwb = nc.dram_tensor("projwb", [C, C], bf16).ap()
    projbb = nc.dram_tensor("projbb", [1, C], bf16).ap()
    wpb = nc.dram_tensor("wpb", [H, P], bf16).ap()
    # pos bounce: m-parity-packed rows of 128 (two padded-64 p-rows each)
    XPAD = sum(-(-(n1 - n0) * MS // 2 // 16) * 16 for n0, n1 in N_RANGES)
    posb = nc.dram_tensor("posb", [XPAD, 128], bf16).ap()
    pos8_loc = nc.dram_tensor("pos8_loc", [H, XMM_P], fp8).ap()
    pos8_all = nc.dram_tensor(
        "pos8_all", [NCORES, H, XMM_P], fp8, addr_space="Shared"
    ).ap()

    with tile.TileContext(nc) as tc:
        kernel_body(
            nc, tc, x_in, qkvw_in, pos_in, wp_in, projw_in, projb_in, y_out,
            xb, qkvwb, projwb, projbb, wpb, posb, pos8_loc, pos8_all,
            sim_mode=sim_mode,
        )
    nc.compile()
    return nc


def kernel_body(nc, tc, x_in, qkvw_in, pos_in, wp_in, projw_in, projb_in,
                y_out, xb, qkvwb, projwb, projbb, wpb, posb, pos8_loc,
                pos8_all, sim_mode=False):
    from contextlib import ExitStack

    ms_last = N - 7 * MS  # 85 valid rows in the last shard

    with ExitStack() as stk:
        const = stk.enter_context(tc.tile_pool(name="const", bufs=1))
        wTd = const.tile([128, H], bf16)       # pos_proj_w.T at bases 0 and 64
        ones_mm = const.tile([1, 128], bf16)   # lhsT for bias broadcast
        projb_sb = const.tile([1, C], bf16)
        ident8 = const.tile([MS, MS], fp8)
        make_identity(nc, ident8[:, :])
        nc.any.memset(ones_mm[:], 1.0)

        # ============ phase 0: pos pipeline -> AllGather ==================
        # cast pos_emb slice f32->bf16 into padded bounce rows
        nc.gpsimd.dma_start(out=wpb[:, :], in_=wp_in[:, :])  # cast H x P
        nc.sync.dma_start(out=wTd[0:P, :], in_=wpb.rearrange("h p -> p h"))
        nc.sync.dma_start(out=wTd[64:64 + P, :], in_=wpb.rearrange("h p -> p h"))

        row0 = 0
        chunk_info = []  # (row0, rows_pad, n0, n1)
        for n0, n1 in N_RANGES:
            rows = (n1 - n0) * MS // 2
            rows_pad = -(-rows // 16) * 16
            nc.gpsimd.dma_start(
                out=posb[row0:row0 + rows, :],
                in_=pos_in[n0:n1].rearrange("n m p -> (n m) p")
                .rearrange("(r two) p -> r (two p)", two=2),
            )
            chunk_info.append((row0, rows_pad, n0, n1))
            row0 += rows_pad

        # ============ phase 1: weight/x prep (overlaps AllGather) =========
        nc.gpsimd.dma_start(out=qkvwb[:, :], in_=qkvw_in[:, :])
        nc.gpsimd.dma_start(out=projwb[:, :], in_=projw_in[:, :])
        nc.gpsimd.dma_start(out=projbb[0, :], in_=projb_in[:])
        nc.sync.dma_start(out=projb_sb[:, :], in_=projbb[:, :])
        nc.gpsimd.dma_start(
            out=xb[0:TOK, :], in_=x_in.rearrange("b n c -> (b n) c"))
        zpad = const.tile([TOKP - TOK, C], bf16)
        nc.any.memset(zpad[:], 0.0)
        nc.sync.dma_start(out=xb[TOK:TOKP, :], in_=zpad[:, :])

        wpool = stk.enter_context(tc.tile_pool(name="weights", bufs=1))
        qkvwT = []   # 6 tiles [128, 2304]
        projwT = []  # 6 tiles [128, 768]
        xT = []      # 6 tiles [128, TOKP]
        for c in range(CK):
            t = wpool.tile([128, 3 * C], bf16, tag=f"qkvwT{c}", name=f"qkvwT{c}")
            nc.sync.dma_start(
                out=t[:, :], in_=qkvwb[:, c * 128:(c + 1) * 128],
                transpose=True)
            qkvwT.append(t)
            t = wpool.tile([128, C], bf16, tag=f"projwT{c}", name=f"projwT{c}")
            nc.sync.dma_start(
                out=t[:, :], in_=projwb[:, c * 128:(c + 1) * 128],
                transpose=True)
            projwT.append(t)
            t = wpool.tile([128, TOKP], bf16, tag=f"xT{c}", name=f"xT{c}")
            nc.sync.dma_start(
                out=t[:, :], in_=xb[:, c * 128:(c + 1) * 128], transpose=True)
            xT.append(t)

        with ExitStack() as pstk:
            ppool = pstk.enter_context(tc.tile_pool(name="posT", bufs=2))
            ppsum = pstk.enter_context(
                tc.tile_pool(name="pos_ps", bufs=4, space="PSUM"))
            pacc = pstk.enter_context(tc.tile_pool(name="pos_acc", bufs=2))

            for row0, rows_pad, n0, n1 in chunk_info:
                nr = n1 - n0
                pT = ppool.tile([128, rows_pad], bf16, tag="posT", name="posT")
                nc.sync.dma_start(
                    out=pT[:, :], in_=posb[row0:row0 + rows_pad, :],
                    transpose=True,
                )
                # per parity: [p, m2, n] views of this n-range
                pTe = pT[0:P, 0:nr * MS // 2].rearrange(
                    "p (n m2) -> p m2 n", m2=MS // 2)
                pTo = pT[64:64 + P, 0:nr * MS // 2].rearrange(
                    "p (n m2) -> p m2 n", m2=MS // 2)
                for mg in range(10):          # 100 = 10 groups of 10
                    acc = pacc.tile([H, 10 * N], fp8, tag="pacc", name="pacc")
                    for mi in range(10):
                        m = mg * 10 + mi
                        par, m2 = m % 2, m // 2
                        ps = ppsum.tile([H, 512], f32, tag="pps", name="pps")
                        nc.tensor.matmul(
                            ps[:, 0:nr],
                            wTd[64 * par:64 * par + P, :],
                            (pTo if par else pTe)[:, m2, :],
                            start=True, stop=True,
                        )
                        if mi % 2 == 0:
                            nc.scalar.activation(
                                acc[:, mi * N + n0: mi * N + n1],
                                ps[:, 0:nr], Copy)
                        else:
                            nc.vector.tensor_copy(
                                acc[:, mi * N + n0: mi * N + n1], ps[:, 0:nr])
                    nc.sync.dma_start(
                        out=pos8_loc[:, mg * 10 * N:(mg + 1) * 10 * N]
                        .rearrange("h (m n) -> h m n", m=10)[:, :, n0:n1],
                        in_=acc[:, 0:10 * N]
                        .rearrange("h (m n) -> h m n", m=10)[:, :, n0:n1],
                    )

        if sim_mode:
            # timing stand-in for the AllGather (sim is single-core)
            for r in range(NCORES):
                nc.sync.dma_start(out=pos8_all[r], in_=pos8_loc[:, :])
        else:
            nc.gpsimd.collective_compute(
                "AllGather", mybir.AluOpType.bypass,
                replica_groups=[list(range(NCORES))],
                ins=[pos8_loc[:, :]], outs=[pos8_all[:, :, :]],
            )

        # ============ phase 2: qkv projection =============================
        qkT = []  # 12 tiles [128, TOK] rows of (q;k).T
        vag = {}  # (b, r) -> [ms, H*(HD+1)] v with ones column
        with ExitStack() as qstk:
            qpool = qstk.enter_context(tc.tile_pool(name="qkv_sb", bufs=1))
            qpsum_stk = ExitStack()
            qpsum = qpsum_stk.enter_context(
                tc.tile_pool(name="qkv_ps", bufs=2, space="PSUM"))
            for mo in range(12):
                t = qpool.tile([128, TOK], bf16, tag=f"qkT{mo}", name=f"qkT{mo}")
                for j0 in range(0, TOK, 512):
                    j1 = min(j0 + 512, TOK)
                    ps = qpsum.tile([128, 512], f32, tag="qk_ps", name="qk_ps")
                    for c in range(CK):
                        nc.tensor.matmul(
                            ps[:, 0:j1 - j0],
                            qkvwT[c][:, mo * 128:(mo + 1) * 128],
                            xT[c][:, j0:j1],
                            start=(c == 0), stop=(c == CK - 1),
                        )
                    nc.scalar.activation(t[:, j0:j1], ps[:, 0:j1 - j0], Copy)
                qkT.append(t)
            # v with the ones column appended per head: [tok, H, HD+1]
            for b in range(BL):
                for r in range(8):
                    ms = MS if r < 7 else ms_last
                    vt = qpool.tile([MS, H * (HD + 1)], bf16, tag=f"vag{b}_{r}", name=f"vag{b}_{r}")
                    nc.any.memset(vt[:], 1.0)
                    t0 = b * N + r * MS
                    for half in range(2):  # v cols 1536:2048, 2048:2304
                        ps = qpsum.tile([MS, 512], f32, tag="v_ps", name="v_ps")
                        w0 = 1536 + half * 512
                        w1 = min(w0 + 512, 2304)
                        for c in range(CK):
                            nc.tensor.matmul(
                                ps[0:ms, 0:w1 - w0],
                                xT[c][:, t0:t0 + ms],
                                qkvwT[c][:, w0:w1],
                                start=(c == 0), stop=(c == CK - 1),
                            )
                        hh0 = half * 8
                        nhh = (w1 - w0) // HD
                        nc.scalar.activation(
                            vt[0:ms].rearrange("m (h d) -> m h d", h=H)
                            [:, hh0:hh0 + nhh, 0:HD],
                            ps[0:ms, 0:w1 - w0].rearrange(
                                "m (h d) -> m h d", d=HD),
                            Copy,
                        )
                    vag[(b, r)] = vt

            qpsum_stk.close()

            # ============ phase 3: attention ==============================
            apool = qstk.enter_context(tc.tile_pool(name="attn_sb", bufs=1))
            aoT = {}  # (b, ct) -> [128, N] bf16 attn_out.T
            for b in range(BL):
                for ct in range(CK):
                    aoT[(b, ct)] = apool.tile([128, N], bf16, tag=f"aoT{b}_{ct}", name=f"aoT{b}_{ct}")

            dpool = qstk.enter_context(tc.tile_pool(name="attn_dyn", bufs=2))
            p8pool = qstk.enter_context(tc.tile_pool(name="p8", bufs=10))
            apsum_stk = ExitStack()
            spsum = apsum_stk.enter_context(
                tc.tile_pool(name="s_ps", bufs=1, space="PSUM"))
            opsum = apsum_stk.enter_context(
                tc.tile_pool(name="o_ps", bufs=1, space="PSUM"))

            for h in range(12):
                kt = qkT[6 + h // 2]
                ko = 64 * (h % 2)
                qt = qkT[h // 2]
                qo = 64 * (h % 2)
                p8 = []
                for r in range(8):
                    ms = MS if r < 7 else ms_last
                    t = p8pool.tile([MS, N], fp8, tag="p8t", name="p8t")
                    nc.sync.dma_start(
                        out=t[0:ms, :],
                        in_=pos8_all[r, h, 0:ms * N]
                        .rearrange("(m n) -> m n", n=N),
                    )
                    p8.append(t)
                po = {}
                for b in range(BL):
                    po[b] = opsum.tile([HD + 1, N], f32, tag=f"o_ps{b}",
                                       name=f"o_ps{b}")
                for r in range(8):
                    ms = MS if r < 7 else ms_last
                    ps, ut, pb = {}, {}, {}
                    for b in range(BL):
                        m0 = b * N + r * MS
                        ps[b] = spsum.tile([MS, N], f32, tag=f"s_ps{b}",
                                           name=f"s_ps{b}")
                        _mm_chunks(
                            nc, ps[b][0:ms],
                            kt[ko:ko + HD, m0:m0 + ms],
                            qt[qo:qo + HD, b * N:(b + 1) * N],
                            start=True, stop=False,
                        )
                        _mm_chunks(
                            nc, ps[b][0:ms],
                            ident8[0:ms, 0:ms],
                            p8[r][0:ms],
                            start=False, stop=True,
                        )
                    for b in range(BL):
                        pb[b] = dpool.tile([MS, N], bf16, tag=f"probsT{b}",
                                           name=f"probsT{b}")
                        nc.scalar.activation(
                            pb[b][0:ms], ps[b][0:ms], Exp, scale=SCALE)
                    for b in range(BL):
                        _mm_chunks(
                            nc, po[b],
                            vag[(b, r)][0:ms]
                            .rearrange("m (h d) -> m h d", h=H)[:, h, :],
                            pb[b][0:ms],
                            start=(r == 0), stop=(r == 7),
                        )
                for b in range(BL):
                    rec = dpool.tile([1, N], f32, tag="recip", name="recip")
                    nc.vector.reciprocal(rec[:, :], po[b][HD:HD + 1, :])
                    recb = dpool.tile([HD, N], f32, tag="recb", name="recb")
                    nc.gpsimd.partition_broadcast(recb[:, :], rec[:, :])
                    ct, co = (h * HD) // 128, (h * HD) % 128
                    nc.vector.tensor_mul(
                        aoT[(b, ct)][co:co + HD, :], po[b][0:HD, :], recb[:, :])

            apsum_stk.close()

            # ============ phase 4: output projection ======================
            ypsum = qstk.enter_context(
                tc.tile_pool(name="y_ps", bufs=2, space="PSUM"))
            ypool = qstk.enter_context(tc.tile_pool(name="y_sb", bufs=2))
            for b in range(BL):
                for to in range(7):
                    t0 = to * 128
                    t1 = min(t0 + 128, N)
                    tw = t1 - t0
                    ps = ypsum.tile([128, C], f32, tag="y_ps", name="y_ps")
                    for j0 in (0, 512):
                        j1 = min(j0 + 512, C)
                        for c in range(CK):
                            nc.tensor.matmul(
                                ps[0:tw, j0:j1],
                                aoT[(b, c)][:, t0:t1],
                                projwT[c][:, j0:j1],
                                start=(c == 0), stop=False,
                            )
                        nc.tensor.matmul(
                            ps[0:tw, j0:j1], ones_mm[:, 0:tw],
                            projb_sb[:, j0:j1], start=False, stop=True,
                        )
                    ys = ypool.tile([128, C], f32, tag="y_sb", name="y_sb")
                    nc.scalar.activation(ys[0:tw], ps[0:tw], Copy)
                    nc.sync.dma_start(
                        out=y_out[b, t0:t1, :], in_=ys[0:tw])


def kernel(**inputs):
    x = np.ascontiguousarray(np.asarray(inputs["x"], dtype=np.float32))
    qkv_w = np.ascontiguousarray(np.asarray(inputs["qkv_w"], np.float32))
    pos_emb = np.ascontiguousarray(np.asarray(inputs["pos_emb"], np.float32))
    wp = np.ascontiguousarray(np.asarray(inputs["pos_proj_w"], np.float32))
    proj_w = np.ascontiguousarray(np.asarray(inputs["proj_w"], np.float32))
    proj_b = np.ascontiguousarray(np.asarray(inputs["proj_b"], np.float32))

    if "nc" not in _cache:
        _cache["nc"] = build()
    nc = _cache["nc"]

    pos_pad = np.zeros((N, NCORES * MS, PP), np.float32)
    pos_pad[:, :N, :P] = pos_emb
    in_maps = []
    for i in range(NCORES):
        in_maps.append({
            "x": np.ascontiguousarray(x[i * BL:(i + 1) * BL]),
            "qkv_w": qkv_w,
            "pos_slice": np.ascontiguousarray(
                pos_pad[:, i * MS:(i + 1) * MS, :]),
            "pos_proj_w": wp,
            "proj_w": proj_w,
            "proj_b": proj_b,
        })
    res = run_bass_kernel_spmd(nc, in_maps, core_ids=list(range(NCORES)))
    _cache["last_res"] = res
    out = np.concatenate([res.results[i]["out"] for i in range(NCORES)], axis=0)
    return out.astype(np.float32)


if __name__ == "__main__":
    import reference
    inp = {k: np.asarray(v) for k, v in reference.setup_inputs().items()}
    got = kernel(**inp)
    exp = np.asarray(reference.reference(**inp))
    err = np.abs(got - exp).max() / (np.abs(exp).max() + 1e-9)
    print("rel err:", err)


# revision 26
# speedup vs baseline: 144.8369x; 1.0171x over previous
"""Trainium2 Bass kernel for nn_Attention_28862180229481.

Attention with learned relative-position bias:
  qkv = x @ qkv_w.T ; q,k,v per head
  pos = einsum('nmp,hp->hnm', pos_emb, pos_proj_w)
  attn = softmax((q@k.T + pos) * scale); out = (attn @ v) @ proj_w.T + proj_b

Sharding: data-parallel over batch (16 batches -> 8 cores x 2).
pos bias is m-sharded: core r computes pos[:, :, r*99:(r+1)*99] (via a
DMA-xbar transpose of pos_emb into [p, n*m] layout + K=48 matmul), stores
it unscaled as fp8e5, AllGathers across the 8 cores, and every core then
consumes the full [12,785,785] bias in fp8 during its local attention.

Softmax: logits are bounded (~N(0,0.31) after scale) so no max-subtraction:
probs = exp(scale*(qk+pos)); row-sum comes free via a ones-column packed
next to V in the attn@v matmul; normalization folds into the PSUM eviction.
"""

import numpy as np

import concourse.bass as bass
import concourse.mybir as mybir
import concourse.tile as tile
from concourse import bacc
from concourse.bass_utils import run_bass_kernel_spmd
from concourse.masks import make_identity

# problem shapes
B, N, C, H, HD, P = 16, 785, 768, 12, 64, 48
NCORES = 8
BL = B // NCORES          # 2 local batches
TOK = BL * N              # 1570
TOKP = 1600               # padded tokens for xbar transpose (mult of 32)
MS = 100                  # m-shard size (8*100 = 800 >= 785)
PP = 64                   # host-padded p dim (48 -> 64)
SCALE = HD ** -0.5
CK = C // 128             # 6 contraction chunks of 128
XMM = MS * N              # 78500 m-major flat size of one pos shard
XMM_P = 78848             # padded to mult of 512 for the collective
# n-range chunks for the pos pipeline (posembT SBUF residency = 99*nr elems)
N_RANGES = [(0, 392), (392, 785)]

f32 = mybir.dt.float32
bf16 = mybir.dt.bfloat16
fp16 = mybir.dt.float16
fp8 = mybir.dt.float8e5
Exp = mybir.ActivationFunctionType.Exp
Copy = mybir.ActivationFunctionType.Copy
ADD = mybir.AluOpType.add

_cache = {}


def _mm_chunks(nc, psum, lhsT, rhs, start, stop, fmax=512):
    """matmul split along the moving free dim into <=512 chunks."""
    F = rhs.shape[-1]
    j = 0
    while j < F:
        je = min(j + fmax, F)
        nc.tensor.matmul(psum[:, j:je], lhsT, rhs[:, j:je], start=start, stop=stop)
        j = je


def build(sim_mode=False):
    nc = bacc.Bacc(
        "TRN2", target_bir_lowering=False, debug=False, num_devices=NCORES
    )

    # ---- I/O -------------------------------------------------------------
    x_in = nc.dram_tensor("x", [BL, N, C], f32, kind="ExternalInput").ap()
    qkvw_in = nc.dram_tensor("qkv_w", [3 * C, C], f32, kind="ExternalInput").ap()
    pos_in = nc.dram_tensor("pos_slice", [N, MS, PP], f32, kind="ExternalInput").ap()
    wp_in = nc.dram_tensor("pos_proj_w", [H, P], f32, kind="ExternalInput").ap()
    projw_in = nc.dram_tensor("proj_w", [C, C], f32, kind="ExternalInput").ap()
    projb_in = nc.dram_tensor("proj_b", [C], f32, kind="ExternalInput").ap()
    y_out = nc.dram_tensor("out", [BL, N, C], f32, kind="ExternalOutput").ap()

    # ---- internal DRAM ---------------------------------------------------
    xb = nc.dram_tensor("xb", [TOKP, C], bf16).ap()            # bf16 x
    qkvwb = nc.dram_tensor("qkvwb", [3 * C, C], bf16).ap()
    projwb = nc.dram_tensor("projwb", [C, C], bf16).ap()
    projbb = nc.dram_tensor("projbb", [1, C], bf16).ap()
    wpb = nc.dram_tensor("wpb", [H, P], bf16).ap()
    # pos bounce: m-parity-packed rows of 128 (two padded-64 p-rows each)
    XPAD = sum(-(-(n1 - n0) * MS // 2 // 16) * 16 for n0, n1 in N_RANGES)
    posb = nc.dram_tensor("posb", [XPAD, 128], bf16).ap()
    pos8_loc = nc.dram_tensor("pos8_loc", [H, XMM_P], fp8).ap()
    pos8_all = nc.dram_tensor(
        "pos8_all", [NCORES, H, XMM_P], fp8, addr_space="Shared"
    ).ap()

    with tile.TileContext(nc) as tc:
        kernel_body(
            nc, tc, x_in, qkvw_in, pos_in, wp_in, projw_in, projb_in, y_out,
            xb, qkvwb, projwb, projbb, wpb, posb, pos8_loc, pos8_all,
            sim_mode=sim_mode,
        )
    nc.compile()
    return nc


def kernel_body(nc, tc, x_in, qkvw_in, pos_in, wp_in, projw_in, projb_in,
                y_out, xb, qkvwb, projwb, projbb, wpb, posb, pos8_loc,
                pos8_all, sim_mode=False):
    from contextlib import ExitStack

    ms_last = N - 7 * MS  # 85 valid rows in the last shard

    with ExitStack() as stk:
        const = stk.enter_context(tc.tile_pool(name="const", bufs=1))
        wTd = const.tile([128, H], bf16)       # pos_proj_w.T at bases 0 and 64
        ones_mm = const.tile([1, 128], bf16)   # lhsT for bias broadcast
        projb_sb = const.tile([1, C], bf16)
        ident8 = const.tile([MS, MS], fp8)
        make_identity(nc, ident8[:, :])
        identb = const.tile([128, 128], bf16)
        make_identity(nc, identb[:, :])
        nc.any.memset(ones_mm[:], 1.0)

        # ============ phase 0: pos pipeline -> AllGather ==================
        # cast pos_emb slice f32->bf16 into padded bounce rows
        nc.gpsimd.dma_start(out=wpb[:, :], in_=wp_in[:, :])  # cast H x P
        nc.sync.dma_start(out=wTd[0:P, :], in_=wpb.rearrange("h p -> p h"))
        nc.sync.dma_start(out=wTd[64:64 + P, :], in_=wpb.rearrange("h p -> p h"))

        row0 = 0
        chunk_info = []  # (row0, rows_pad, n0, n1)
        for n0, n1 in N_RANGES:
            rows = (n1 - n0) * MS // 2
            rows_pad = -(-rows // 16) * 16
            nc.gpsimd.dma_start(
                out=posb[row0:row0 + rows, :],
                in_=pos_in[n0:n1].rearrange("n m p -> (n m) p")
                .rearrange("(r two) p -> r (two p)", two=2),
            )
            chunk_info.append((row0, rows_pad, n0, n1))
            row0 += rows_pad

        # ============ phase 1: weight/x prep (overlaps AllGather) =========
        nc.gpsimd.dma_start(out=projbb[0, :], in_=projb_in[:])
        nc.sync.dma_start(out=projb_sb[:, :], in_=projbb[:, :])
        nc.gpsimd.dma_start(
            out=xb[0:TOK, :], in_=x_in.rearrange("b n c -> (b n) c"))
        zpad = const.tile([TOKP - TOK, C], bf16)
        nc.any.memset(zpad[:], 0.0)
        nc.sync.dma_start(out=xb[TOK:TOKP, :], in_=zpad[:, :])

        wpool = stk.enter_context(tc.tile_pool(name="weights", bufs=1))
        qkvwT = []   # 6 tiles [128, 2304]
        projwT = []  # 6 tiles [128, 768]
        xT = []      # 6 tiles [128, TOKP]
        for c in range(CK):
            t = wpool.tile([128, 3 * C], bf16, tag=f"qkvwT{c}", name=f"qkvwT{c}")
            qkvwT.append(t)
            t = wpool.tile([128, C], bf16, tag=f"projwT{c}", name=f"projwT{c}")
            projwT.append(t)
            t = wpool.tile([128, TOKP], bf16, tag=f"xT{c}", name=f"xT{c}")
            nc.sync.dma_start(
                out=t[:, :], in_=xb[:, c * 128:(c + 1) * 128], transpose=True)
            xT.append(t)
        # transpose qkv_w / proj_w on the (idle) TensorEngine instead of DMA
        with ExitStack() as wstk:
            wfp = wstk.enter_context(tc.tile_pool(name="wf", bufs=3))
            tps = wstk.enter_context(
                tc.tile_pool(name="tp_ps", bufs=4, space="PSUM"))
            for src_ap, dst, tot in ((qkvw_in, qkvwT, 3 * C),
                                     (projw_in, projwT, C)):
                for ro in range(-(-tot // 128)):
                    rows = min(128, tot - ro * 128)
                    wf = wfp.tile([128, C], f32, tag="wf", name="wf")
                    nc.sync.dma_start(
                        out=wf[0:rows, :],
                        in_=src_ap[ro * 128:ro * 128 + rows, :])
                    wb16 = wfp.tile([128, C], bf16, tag="wb16", name="wb16")
                    nc.vector.tensor_copy(wb16[0:rows, :], wf[0:rows, :])
                    for c in range(CK):
                        tp = tps.tile([128, 128], bf16, tag="tp", name="tp")
                        nc.tensor.transpose(
                            tp[:, 0:rows],
                            wb16[0:rows, c * 128:(c + 1) * 128],
                            identb[0:rows, 0:rows])
                        nc.scalar.activation(
                            dst[c][:, ro * 128:ro * 128 + rows],
                            tp[:, 0:rows], Copy)

        with ExitStack() as pstk:
            ppool = pstk.enter_context(tc.tile_pool(name="posT", bufs=2))
            ppsum = pstk.enter_context(
                tc.tile_pool(name="pos_ps", bufs=4, space="PSUM"))
            pacc = pstk.enter_context(tc.tile_pool(name="pos_acc", bufs=3))

            for row0, rows_pad, n0, n1 in chunk_info:
                nr = n1 - n0
                pT = ppool.tile([128, rows_pad], bf16, tag="posT", name="posT")
                nc.sync.dma_start(
                    out=pT[:, :], in_=posb[row0:row0 + rows_pad, :],
                    transpose=True,
                )
                # per parity: [p, m2, n] views of this n-range
                pTe = pT[0:P, 0:nr * MS // 2].rearrange(
                    "p (n m2) -> p m2 n", m2=MS // 2)
                pTo = pT[64:64 + P, 0:nr * MS // 2].rearrange(
                    "p (n m2) -> p m2 n", m2=MS // 2)
                for mg in range(10):          # 100 = 10 groups of 10
                    acc = pacc.tile([H, 10 * N], fp8, tag="pacc", name="pacc")
                    for mi in range(10):
                        m = mg * 10 + mi
                        par, m2 = m % 2, m // 2
                        ps = ppsum.tile([H, 512], f32, tag="pps", name="pps")
                        nc.tensor.matmul(
                            ps[:, 0:nr],
                            wTd[64 * par:64 * par + P, :],
                            (pTo if par else pTe)[:, m2, :],
                            start=True, stop=True,
                        )
                        if mi % 2 == 0:
                            nc.scalar.activation(
                                acc[:, mi * N + n0: mi * N + n1],
                                ps[:, 0:nr], Copy)
                        else:
                            nc.vector.tensor_copy(
                                acc[:, mi * N + n0: mi * N + n1], ps[:, 0:nr])
                    nc.sync.dma_start(
                        out=pos8_loc[:, mg * 10 * N:(mg + 1) * 10 * N]
                        .rearrange("h (m n) -> h m n", m=10)[:, :, n0:n1],
                        in_=acc[:, 0:10 * N]
                        .rearrange("h (m n) -> h m n", m=10)[:, :, n0:n1],
                    )

        if sim_mode:
            # timing stand-in for the AllGather (sim is single-core)
            for r in range(NCORES):
                nc.sync.dma_start(out=pos8_all[r], in_=pos8_loc[:, :])
        else:
            nc.gpsimd.collective_compute(
                "AllGather", mybir.AluOpType.bypass,
                replica_groups=[list(range(NCORES))],
                ins=[pos8_loc[:, :]], outs=[pos8_all[:, :, :]],
            )

        # ============ phase 2: qkv projection =============================
        qkT = []  # 12 tiles [128, TOK] rows of (q;k).T
        vag = {}  # (b, r) -> [ms, H*(HD+1)] v with ones column
        with ExitStack() as qstk:
            qpool = qstk.enter_context(tc.tile_pool(name="qkv_sb", bufs=1))
            qpsum_stk = ExitStack()
            qpsum = qpsum_stk.enter_context(
                tc.tile_pool(name="qkv_ps", bufs=2, space="PSUM"))
            for mo in range(12):
                t = qpool.tile([128, TOK], bf16, tag=f"qkT{mo}", name=f"qkT{mo}")
                for j0 in range(0, TOK, 512):
                    j1 = min(j0 + 512, TOK)
                    ps = qpsum.tile([128, 512], f32, tag="qk_ps", name="qk_ps")
                    for c in range(CK):
                        nc.tensor.matmul(
                            ps[:, 0:j1 - j0],
                            qkvwT[c][:, mo * 128:(mo + 1) * 128],
                            xT[c][:, j0:j1],
                            start=(c == 0), stop=(c == CK - 1),
                        )
                    nc.scalar.activation(t[:, j0:j1], ps[:, 0:j1 - j0], Copy)
                qkT.append(t)
            # v with the ones column appended per head: [tok, H, HD+1]
            for b in range(BL):
                for r in range(8):
                    ms = MS if r < 7 else ms_last
                    vt = qpool.tile([MS, H * (HD + 1)], bf16, tag=f"vag{b}_{r}", name=f"vag{b}_{r}")
                    nc.any.memset(vt[:], 1.0)
                    t0 = b * N + r * MS
                    for half in range(2):  # v cols 1536:2048, 2048:2304
                        ps = qpsum.tile([MS, 512], f32, tag="v_ps", name="v_ps")
                        w0 = 1536 + half * 512
                        w1 = min(w0 + 512, 2304)
                        for c in range(CK):
                            nc.tensor.matmul(
                                ps[0:ms, 0:w1 - w0],
                                xT[c][:, t0:t0 + ms],
                                qkvwT[c][:, w0:w1],
                                start=(c == 0), stop=(c == CK - 1),
                            )
                        hh0 = half * 8
                        nhh = (w1 - w0) // HD
                        nc.scalar.activation(
                            vt[0:ms].rearrange("m (h d) -> m h d", h=H)
                            [:, hh0:hh0 + nhh, 0:HD],
                            ps[0:ms, 0:w1 - w0].rearrange(
                                "m (h d) -> m h d", d=HD),
                            Copy,
                        )
                    vag[(b, r)] = vt

            qpsum_stk.close()

            # ============ phase 3: attention ==============================
            apool = qstk.enter_context(tc.tile_pool(name="attn_sb", bufs=1))
            aoT = {}  # (b, ct) -> [128, N] bf16 attn_out.T
            for b in range(BL):
                for ct in range(CK):
                    aoT[(b, ct)] = apool.tile([128, N], bf16, tag=f"aoT{b}_{ct}", name=f"aoT{b}_{ct}")

            dpool = qstk.enter_context(tc.tile_pool(name="attn_dyn", bufs=3))
            p8pool = qstk.enter_context(tc.tile_pool(name="p8", bufs=16))
            apsum_stk = ExitStack()
            spsum = apsum_stk.enter_context(
                tc.tile_pool(name="s_ps", bufs=1, space="PSUM"))
            opsum = apsum_stk.enter_context(
                tc.tile_pool(name="o_ps", bufs=1, space="PSUM"))

            for h in range(12):
                kt = qkT[6 + h // 2]
                ko = 64 * (h % 2)
                qt = qkT[h // 2]
                qo = 64 * (h % 2)
                p8 = []
                for r in range(8):
                    ms = MS if r < 7 else ms_last
                    t = p8pool.tile([MS, N], fp8, tag="p8t", name="p8t")
                    nc.sync.dma_start(
                        out=t[0:ms, :],
                        in_=pos8_all[r, h, 0:ms * N]
                        .rearrange("(m n) -> m n", n=N),
                    )
                    p8.append(t)
                po = {}
                for b in range(BL):
                    po[b] = opsum.tile([HD + 1, N], f32, tag=f"o_ps{b}",
                                       name=f"o_ps{b}")
                for r in range(8):
                    ms = MS if r < 7 else ms_last
                    ps, ut, pb = {}, {}, {}
                    for b in range(BL):
                        m0 = b * N + r * MS
                        ps[b] = spsum.tile([MS, N], f32, tag=f"s_ps{b}",
                                           name=f"s_ps{b}")
                        _mm_chunks(
                            nc, ps[b][0:ms],
                            kt[ko:ko + HD, m0:m0 + ms],
                            qt[qo:qo + HD, b * N:(b + 1) * N],
                            start=True, stop=False,
                        )
                        _mm_chunks(
                            nc, ps[b][0:ms],
                            ident8[0:ms, 0:ms],
                            p8[r][0:ms],
                            start=False, stop=True,
                        )
                    for b in range(BL):
                        pb[b] = dpool.tile([MS, N], bf16, tag=f"probsT{b}",
                                           name=f"probsT{b}")
                        nc.scalar.activation(
                            pb[b][0:ms], ps[b][0:ms], Exp, scale=SCALE)
                    for b in range(BL):
                        _mm_chunks(
                            nc, po[b],
                            vag[(b, r)][0:ms]
                            .rearrange("m (h d) -> m h d", h=H)[:, h, :],
                            pb[b][0:ms],
                            start=(r == 0), stop=(r == 7),
                        )
                for b in range(BL):
                    rec = dpool.tile([1, N], f32, tag="recip", name="recip")
                    nc.vector.reciprocal(rec[:, :], po[b][HD:HD + 1, :])
                    recb = dpool.tile([HD, N], f32, tag="recb", name="recb")
                    nc.gpsimd.partition_broadcast(recb[:, :], rec[:, :])
                    ct, co = (h * HD) // 128, (h * HD) % 128
                    nc.vector.tensor_mul(
                        aoT[(b, ct)][co:co + HD, :], po[b][0:HD, :], recb[:, :])

            apsum_stk.close()

            # ============ phase 4: output projection ======================
            ypsum = qstk.enter_context(
                tc.tile_pool(name="y_ps", bufs=2, space="PSUM"))
            ypool = qstk.enter_context(tc.tile_pool(name="y_sb", bufs=2))
            for b in range(BL):
                for to in range(7):
                    t0 = to * 128
                    t1 = min(t0 + 128, N)
                    tw = t1 - t0
                    ps = ypsum.tile([128, C], f32, tag="y_ps", name="y_ps")
                    for j0 in (0, 512):
                        j1 = min(j0 + 512, C)
                        for c in range(CK):
                            nc.tensor.matmul(
                                ps[0:tw, j0:j1],
                                aoT[(b, c)][:, t0:t1],
                                projwT[c][:, j0:j1],
                                start=(c == 0), stop=False,
                            )
                        nc.tensor.matmul(
                            ps[0:tw, j0:j1], ones_mm[:, 0:tw],
                            projb_sb[:, j0:j1], start=False, stop=True,
                        )
                    ys = ypool.tile([128, C], f32, tag="y_sb", name="y_sb")
                    nc.scalar.activation(ys[0:tw], ps[0:tw], Copy)
                    nc.sync.dma_start(
                        out=y_out[b, t0:t1, :], in_=ys[0:tw])


def kernel(**inputs):
    x = np.ascontiguousarray(np.asarray(inputs["x"], dtype=np.float32))
    qkv_w = np.ascontiguousarray(np.asarray(inputs["qkv_w"], np.float32))
    pos_emb = np.ascontiguousarray(np.asarray(inputs["pos_emb"], np.float32))
    wp = np.ascontiguousarray(np.asarray(inputs["pos_proj_w"], np.float32))
    proj_w = np.ascontiguousarray(np.asarray(inputs["proj_w"], np.float32))
    proj_b = np.ascontiguousarray(np.asarray(inputs["proj_b"], np.float32))

    if "nc" not in _cache:
        _cache["nc"] = build()
    nc = _cache["nc"]

    pos_pad = np.zeros((N, NCORES * MS, PP), np.float32)
    pos_pad[:, :N, :P] = pos_emb
    in_maps = []
    for i in range(NCORES):
        in_maps.append({
            "x": np.ascontiguousarray(x[i * BL:(i + 1) * BL]),
            "qkv_w": qkv_w,
            "pos_slice": np.ascontiguousarray(
                pos_pad[:, i * MS:(i + 1) * MS, :]),
            "pos_proj_w": wp,
            "proj_w": proj_w,
            "proj_b": proj_b,
        })
    res = run_bass_kernel_spmd(nc, in_maps, core_ids=list(range(NCORES)))
    _cache["last_res"] = res
    out = np.concatenate([res.results[i]["out"] for i in range(NCORES)], axis=0)
    return out.astype(np.float32)


if __name__ == "__main__":
    import reference
    inp = {k: np.asarray(v) for k, v in reference.setup_inputs().items()}
    got = kernel(**inp)
    exp = np.asarray(reference.reference(**inp))
    err = np.abs(got - exp).max() / (np.abs(exp).max() + 1e-9)
    print("rel err:", err)
